# revision 20
# baseline (speedup 1.0000x reference)
"""DBLoss (OHEM-masked BCE + masked L1 threshold loss) on 8 Trainium2 cores.

Shapes are hardcoded for the nn_DBLoss problem:
  outputs             [16, 3, 640, 640] f32
  gt_shrink_labels    [16, 640, 640]    f32
  gt_threshold_labels [16, 640, 640]    f32
Returns np.float32[4] = (loss_all, loss_shrink, loss_binary, loss_thresh).

Sharding: pure data parallel — 2 images per core, 8 cores. Each core computes
per-image partial sums (per-partition [128] vectors); the host reduces the
tiny partials and forms the masked means.

Math notes (device fast path):
 * OHEM: with neg_num == neg_total (i.e. 3*pos_num >= neg_total) the top-k
   threshold is the minimum negative score, so the selection mask is exactly
   all-ones for every valid image. The host verifies this condition per image
   (along with pos_num>0, neg_total>0) and falls back to an exact numpy
   implementation if any image needs a true top-k (cannot happen for the
   problem's uniform-random labels).
 * BCE with binarized target t and no sigmoid clipping reduces to
   softplus(x) - t*x; the host verifies |logits| < 16 so the 1e-7 clip in the
   reference is inactive.
 * threshold-loss mask (gt_t>0)|(gt_s>0): the device sums over all pixels;
   the host subtracts exact corrections for the (measure-zero) pixels where
   both labels are <= 0.
"""

import sys

import numpy as np

try:
    import concourse.bass as bass
except ImportError:  # stand-alone grading dir: fall back to known repo paths
    for _p in ("/root/.axon_site/_ro/trn_rl_repo", "/opt/trn_rl_repo"):
        if _p not in sys.path:
            sys.path.append(_p)
    import concourse.bass as bass

import concourse.tile as tile
from concourse import mybir
from concourse.bass_utils import run_bass_kernel_spmd

B, H, W = 16, 640, 640
N = H * W                    # 409600 pixels / image
P = 128                      # SBUF partitions
F = N // P                   # 3200 free elements / partition
NCORES = 8
BPC = B // NCORES            # 2 images per core
ALPHA, BETA = 1.0, 10.0
F32 = mybir.dt.float32

_CACHED_NC = None


def build_nc() -> "bass.Bass":
    """Per-core raw-bass program.

    Per image: 5 HWDGE channel loads, 7 ACT table ops (exp/ln set only),
    4 big DVE ops; per-partition partial sums in one output tile.

    Raw bass (no TileContext): this walrus build encodes at most ONE attached
    sync-wait per TPB instruction and Tile's kernel-tail drain needs ~10, so
    all cross-engine ordering uses standalone wait_ge instructions
    (EventSemaphore ops, which codegen fine) with explicit semaphores.

    Load order is tuned so ACT (the busiest engine at ~41.4 us of table ops)
    starts after the first 1.6 MB load and never stalls long, and so the
    last-arriving tensors gate the least trailing work:
      tm0 g0 s0 bn0 tm1 gt0 g1 s1 gt1 bn1

    Semaphores: one per input DMA (+16 on completion), sa = ACT op counter
    (then_inc fires on write-ack, so sa>=k also guards same-engine RAW/WAW
    on ACT outputs), sv = DVE op counter, sc = bias-constant memset done,
    dout = output DMA completion. DVE clears every semaphore at the end so
    repeated executions of the loaded NEFF start from zero.
    """
    nc = bass.Bass(dynamic_dma_scratch_size=2048, enable_partition_id=False,
                   monotonic_sem_count=0)
    outs = nc.dram_tensor("outs", [BPC, 3, N], F32, kind="ExternalInput")
    gts = nc.dram_tensor("gts", [BPC, N], F32, kind="ExternalInput")
    gtt = nc.dram_tensor("gtt", [BPC, N], F32, kind="ExternalInput")
    # columns per image b: [2b]=sum softplus(shrink), [2b+1]=sum softplus(bin)
    # then [4+3b]=sum t*shrink, [5+3b]=sum t*bin, [6+3b]=sum|sig-gt|
    part = nc.dram_tensor("part", [P, 5 * BPC], F32, kind="ExternalOutput")

    ag = mybir.AluOpType.is_gt
    mul = mybir.AluOpType.mult
    sub = mybir.AluOpType.subtract
    fexp = mybir.ActivationFunctionType.Exp
    fln = mybir.ActivationFunctionType.Ln
    X = mybir.AxisListType.X
    add = mybir.AluOpType.add

    from contextlib import ExitStack
    ctx = ExitStack()
    with ctx:
        sb = lambda nm, shape: ctx.enter_context(nc.sbuf_tensor(nm, shape, F32))
        sem = lambda nm: ctx.enter_context(nc.semaphore(name=nm))
        tm = [sb("tm_0", [P, F]), sb("tm_1", [P, F])]
        s = [sb("s_0", [P, F]), sb("s_1", [P, F])]
        bn = [sb("bn_0", [P, F]), sb("bn_1", [P, F])]
        g = [sb("g_0", [P, F]), sb("g_1", [P, F])]
        gt = [sb("gt_0", [P, F]), sb("gt_1", [P, F])]
        u = [sb("u_0", [P, F]), sb("u_1", [P, F])]
        eu, tr = sb("eu", [P, F]), sb("tr", [P, F])
        po = sb("po", [P, 5 * BPC])
        bias1 = sb("bias1", [P, 1])
        dtm = [sem("dtm0"), sem("dtm1")]
        dtm0b = sem("dtm0b")
        ds = [sem("ds0"), sem("ds1")]
        dbn = [sem("dbn0"), sem("dbn1")]
        dg = [sem("dg0"), sem("dg1")]
        dgt = [sem("dgt0"), sem("dgt1")]
        dout, sa, sv, sc = (sem(nm) for nm in ("dout", "sa", "sv", "sc"))
        all_sems = (dtm + ds + dbn + dg + dgt + [dtm0b, dout, sa, sv, sc])
        block = ctx.enter_context(nc.Block(no_gpsimd_drain=True))

        pf = lambda t: t.rearrange("(p f) -> p f", p=P)

        @block.sync
        def _(sync):
            h = F // 2
            tm0f = pf(outs[0, 1])
            sync.dma_start(out=tm[0][:, :h], in_=tm0f[:, :h]).then_inc(dtm[0], 16)
            sync.dma_start(out=tm[0][:, h:], in_=tm0f[:, h:]).then_inc(dtm0b, 16)
            loads = [
                (s[0], outs[0, 0], ds[0]),
                (g[0], gts[0], dg[0]),
                (bn[0], outs[0, 2], dbn[0]),
                (tm[1], outs[1, 1], dtm[1]),
                (gt[0], gtt[0], dgt[0]),
                (s[1], outs[1, 0], ds[1]),
                (g[1], gts[1], dg[1]),
                (gt[1], gtt[1], dgt[1]),
                (bn[1], outs[1, 2], dbn[1]),
            ]
            for dst, src, dsem in loads:
                sync.dma_start(out=dst[:, :], in_=pf(src)).then_inc(dsem, 16)
            sync.wait_ge(sa, 7 * BPC + 1)
            sync.wait_ge(sv, 4 * BPC)
            sync.dma_start(out=part[:, :], in_=po[:, :]).then_inc(dout, 16)
            for semh in all_sems:
                if semh is not dout:
                    sync.sem_clear(semh)
            sync.wait_ge(dout, 16)
            sync.sem_clear(dout)

        @block.scalar
        def _(scalar):
            h = F // 2
            sa_n = 0

            def act(out, in_, func, wait_prev=True, **kw):
                nonlocal sa_n
                if wait_prev and sa_n >= 1:
                    scalar.wait_ge(sa, sa_n)    # write-ack of previous ACT op
                nc.scalar.activation(out=out, in_=in_, func=func,
                                     **kw).then_inc(sa, 1)
                sa_n += 1

            for b in range(BPC):
                # sigmoid(tm) = exp(-ln(1 + exp(-tm))) in place in u[b];
                # image 0's first exp runs in halves so ACT starts right
                # after the first half-load lands
                if b == 0:
                    scalar.wait_ge(dtm[0], 16)
                    act(u[0][:, :h], tm[0][:, :h], fexp, wait_prev=False,
                        scale=-1.0)
                    scalar.wait_ge(dtm0b, 16)
                    act(u[0][:, h:], tm[0][:, h:], fexp, wait_prev=False,
                        scale=-1.0)
                    scalar.wait_ge(sc, 1)
                    scalar.wait_ge(sa, sa_n)
                else:
                    scalar.wait_ge(dtm[b], 16)
                    act(u[b][:, :], tm[b][:, :], fexp, scale=-1.0)
                act(u[b][:, :], u[b][:, :], fln, bias=bias1[:, :],
                    wait_prev=(b > 0))
                act(u[b][:, :], u[b][:, :], fexp, scale=-1.0)
                # BCE softplus sums: ln(1 + exp(x)), accumulated per partition
                scalar.wait_ge(ds[b], 16)
                act(eu[:, :], s[b][:, :], fexp)
                act(eu[:, :], eu[:, :], fln, bias=bias1[:, :],
                    accum_out=po[:, 2 * b : 2 * b + 1])
                scalar.wait_ge(dbn[b], 16)
                act(eu[:, :], bn[b][:, :], fexp)
                act(eu[:, :], eu[:, :], fln, bias=bias1[:, :],
                    accum_out=po[:, 2 * b + 1 : 2 * b + 2])
            assert sa_n == 7 * BPC + 1

        @block.vector
        def _(vector):
            nc.vector.memset(bias1[:, :], 1.0).then_inc(sc, 1)
            sv_n = 0

            def stt_sum(b, which):
                # sum (g>0.5)*x for x in {s: col c, bn: col c+1}; writes tr
                nonlocal sv_n
                x, dx, off = ((s, ds, 0) if which == "s" else (bn, dbn, 1))
                if sv_n >= 1:
                    vector.wait_ge(sv, sv_n)   # tr write-ack of previous stt
                vector.wait_ge(dg[b], 16)
                vector.wait_ge(dx[b], 16)
                nc.vector.scalar_tensor_tensor(
                    out=tr[:, :], in0=g[b][:, :], scalar=0.5, in1=x[b][:, :],
                    op0=ag, op1=mul,
                    accum_out=po[:, 4 + 3 * b + off : 5 + 3 * b + off],
                ).then_inc(sv, 1)
                sv_n += 1

            def l1_pair(b):
                # |sigmoid - gt| summed: subtract in place into gt, abs-reduce
                nonlocal sv_n
                vector.wait_ge(sa, 7 * b + 4)   # sigmoid chain done
                vector.wait_ge(dgt[b], 16)
                nc.vector.tensor_tensor(
                    out=gt[b][:, :], in0=u[b][:, :], in1=gt[b][:, :], op=sub
                ).then_inc(sv, 1)
                sv_n += 1
                vector.wait_ge(sv, sv_n)        # subtract write-ack
                nc.vector.tensor_reduce(
                    out=po[:, 6 + 3 * b : 7 + 3 * b], in_=gt[b][:, :],
                    axis=X, op=add, apply_absolute_value=True,
                ).then_inc(sv, 1)
                sv_n += 1

            # image 0: bn arrives before gt; image 1: bn arrives last
            stt_sum(0, "s")
            stt_sum(0, "bn")
            l1_pair(0)
            stt_sum(1, "s")
            l1_pair(1)
            stt_sum(1, "bn")
            assert sv_n == 4 * BPC

    return nc


def _numpy_reference(outputs, gt_shrink_labels, gt_threshold_labels):
    """Exact fallback for inputs outside the fast-path regime."""
    OHEM_RATIO, EPS = 3, 1e-7

    def sigmoid(x):
        return 1.0 / (1.0 + np.exp(-x))

    shrink, thresh, binary = outputs[:, 0], outputs[:, 1], outputs[:, 2]
    b = outputs.shape[0]
    flat_s = shrink.reshape(b, -1)
    flat_pos = (gt_shrink_labels > 0.5).reshape(b, -1)
    n = flat_s.shape[1]
    pos_num = flat_pos.sum(axis=1)
    neg_total = n - pos_num
    neg_num = np.minimum(pos_num * OHEM_RATIO, neg_total)
    neg_scores = np.where(flat_pos, -np.inf, flat_s)
    sorted_desc = -np.sort(-neg_scores, axis=1)
    idx = np.clip(neg_num - 1, 0, n - 1).astype(np.int64)
    thr = np.take_along_axis(sorted_desc, idx[:, None], axis=1)
    mask = (flat_s >= thr) | flat_pos
    valid = (pos_num > 0) & (neg_num > 0)
    mask = (mask & valid[:, None]).reshape(shrink.shape).astype(np.float32)

    def masked_bce(logits, target, m):
        p = np.clip(sigmoid(logits), EPS, 1.0 - EPS)
        t = (target > 0.5).astype(np.float32)
        per_px = -(t * np.log(p) + (1.0 - t) * np.log(1.0 - p))
        denom = m.sum()
        return float(per_px.flatten() @ m.flatten() / max(denom, 1.0)) if denom > 0 else 0.0

    loss_shrink = masked_bce(shrink, gt_shrink_labels, mask)
    loss_binary = masked_bce(binary, gt_shrink_labels, mask)
    m2 = ((gt_threshold_labels > 0) | (gt_shrink_labels > 0)).astype(np.float32)
    denom2 = m2.sum()
    l1 = np.abs(sigmoid(thresh) - gt_threshold_labels).flatten() @ m2.flatten()
    loss_thresh = float(l1 / max(denom2, 1.0)) if denom2 > 0 else 0.0
    loss_all = loss_shrink + ALPHA * loss_binary + BETA * loss_thresh
    return np.array([loss_all, loss_shrink, loss_binary, loss_thresh], np.float32)


def kernel(outputs, gt_shrink_labels, gt_threshold_labels, _trace=False):
    global _CACHED_NC
    outputs = np.ascontiguousarray(np.asarray(outputs, dtype=np.float32))
    gts = np.ascontiguousarray(np.asarray(gt_shrink_labels, dtype=np.float32))
    gtt = np.ascontiguousarray(np.asarray(gt_threshold_labels, dtype=np.float32))

    # ---- host-side regime checks (exactness guards for the fast path) ----
    pos_num = (gts > 0.5).reshape(B, -1).sum(axis=1)
    neg_total = N - pos_num
    neg_num = np.minimum(3 * pos_num, neg_total)
    valid = (pos_num > 0) & (neg_num > 0)
    needs_topk = valid & (3 * pos_num < neg_total)
    clip_active = max(
        float(np.abs(outputs[:, 0]).max()), float(np.abs(outputs[:, 2]).max())
    ) >= 16.0
    if needs_topk.any() or clip_active:
        return _numpy_reference(outputs, gts, gtt)

    if _CACHED_NC is None:
        _CACHED_NC = build_nc()
    nc = _CACHED_NC

    in_maps = []
    for c in range(NCORES):
        sl = slice(c * BPC, (c + 1) * BPC)
        in_maps.append({
            "outs": outputs[sl].reshape(BPC, 3, N),
            "gts": gts[sl].reshape(BPC, N),
            "gtt": gtt[sl].reshape(BPC, N),
        })
    res = run_bass_kernel_spmd(
        nc, in_maps, core_ids=list(range(NCORES)), trace=_trace
    )

    # ---- host combine: per-image sums from per-partition partials ----
    sp_s = np.empty(B); sp_b = np.empty(B); ts = np.empty(B); tb = np.empty(B)
    l1 = np.empty(B)
    for c in range(NCORES):
        po = res.results[c]["part"].astype(np.float64).sum(axis=0)
        for b in range(BPC):
            i = c * BPC + b
            sp_s[i], sp_b[i] = po[2 * b], po[2 * b + 1]
            ts[i], tb[i], l1[i] = po[4 + 3 * b], po[5 + 3 * b], po[6 + 3 * b]

    cnt = float(N * valid.sum())
    num_s = float(((sp_s - ts) * valid).sum())
    num_b = float(((sp_b - tb) * valid).sum())
    loss_shrink = num_s / max(cnt, 1.0) if cnt > 0 else 0.0
    loss_binary = num_b / max(cnt, 1.0) if cnt > 0 else 0.0

    # threshold-loss mask corrections for pixels where both labels <= 0
    zz = (gtt <= 0) & (gts <= 0)
    cnt2 = float(B * N - zz.sum())
    l1_tot = float(l1.sum())
    if zz.any():
        tmz = outputs[:, 1][zz]
        l1_tot -= float(np.abs(1.0 / (1.0 + np.exp(-tmz)) - gtt[zz]).sum())
    loss_thresh = l1_tot / max(cnt2, 1.0) if cnt2 > 0 else 0.0

    loss_all = loss_shrink + ALPHA * loss_binary + BETA * loss_thresh
    out = np.array([loss_all, loss_shrink, loss_binary, loss_thresh], np.float32)
    if _trace:
        return out, res
    return out


# revision 21
# speedup vs baseline: 1.0067x; 1.0067x over previous
"""DBLoss (OHEM-masked BCE + masked L1 threshold loss) on 8 Trainium2 cores.

Shapes are hardcoded for the nn_DBLoss problem:
  outputs             [16, 3, 640, 640] f32
  gt_shrink_labels    [16, 640, 640]    f32
  gt_threshold_labels [16, 640, 640]    f32
Returns np.float32[4] = (loss_all, loss_shrink, loss_binary, loss_thresh).

Sharding: pure data parallel — 2 images per core, 8 cores. Each core computes
per-image partial sums (per-partition [128] vectors); the host reduces the
tiny partials and forms the masked means.

Math notes (device fast path):
 * OHEM: with neg_num == neg_total (i.e. 3*pos_num >= neg_total) the top-k
   threshold is the minimum negative score, so the selection mask is exactly
   all-ones for every valid image. The host verifies this condition per image
   (along with pos_num>0, neg_total>0) and falls back to an exact numpy
   implementation if any image needs a true top-k (cannot happen for the
   problem's uniform-random labels).
 * BCE with binarized target t and no sigmoid clipping reduces to
   softplus(x) - t*x; the host verifies |logits| < 16 so the 1e-7 clip in the
   reference is inactive.
 * threshold-loss mask (gt_t>0)|(gt_s>0): the device sums over all pixels;
   the host subtracts exact corrections for the (measure-zero) pixels where
   both labels are <= 0.
"""

import sys

import numpy as np

try:
    import concourse.bass as bass
except ImportError:  # stand-alone grading dir: fall back to known repo paths
    for _p in ("/root/.axon_site/_ro/trn_rl_repo", "/opt/trn_rl_repo"):
        if _p not in sys.path:
            sys.path.append(_p)
    import concourse.bass as bass

import concourse.tile as tile
from concourse import mybir
from concourse.bass_utils import run_bass_kernel_spmd

B, H, W = 16, 640, 640
N = H * W                    # 409600 pixels / image
P = 128                      # SBUF partitions
F = N // P                   # 3200 free elements / partition
NCORES = 8
BPC = B // NCORES            # 2 images per core
ALPHA, BETA = 1.0, 10.0
F32 = mybir.dt.float32

_CACHED_NC = None


def build_nc() -> "bass.Bass":
    """Per-core raw-bass program.

    Per image: 5 HWDGE channel loads, 7 ACT table ops (exp/ln set only),
    4 big DVE ops; per-partition partial sums in one output tile.

    Raw bass (no TileContext): this walrus build encodes at most ONE attached
    sync-wait per TPB instruction and Tile's kernel-tail drain needs ~10, so
    all cross-engine ordering uses standalone wait_ge instructions
    (EventSemaphore ops, which codegen fine) with explicit semaphores.

    Load order is tuned so ACT (the busiest engine at ~41.4 us of table ops)
    starts after the first 1.6 MB load and never stalls long, and so the
    last-arriving tensors gate the least trailing work:
      tm0 g0 s0 bn0 tm1 gt0 g1 s1 gt1 bn1

    Semaphores: one per input DMA (+16 on completion), sa = ACT op counter
    (then_inc fires on write-ack, so sa>=k also guards same-engine RAW/WAW
    on ACT outputs), sv = DVE op counter, sc = bias-constant memset done,
    dout = output DMA completion. DVE clears every semaphore at the end so
    repeated executions of the loaded NEFF start from zero.
    """
    nc = bass.Bass(dynamic_dma_scratch_size=2048, enable_partition_id=False,
                   monotonic_sem_count=0)
    outs = nc.dram_tensor("outs", [BPC, 3, N], F32, kind="ExternalInput")
    gts = nc.dram_tensor("gts", [BPC, N], F32, kind="ExternalInput")
    gtt = nc.dram_tensor("gtt", [BPC, N], F32, kind="ExternalInput")
    # columns per image b: [2b]=sum softplus(shrink), [2b+1]=sum softplus(bin)
    # then [4+3b]=sum t*shrink, [5+3b]=sum t*bin, [6+3b]=sum|sig-gt|
    part = nc.dram_tensor("part", [P, 5 * BPC], F32, kind="ExternalOutput")

    ag = mybir.AluOpType.is_gt
    mul = mybir.AluOpType.mult
    sub = mybir.AluOpType.subtract
    fexp = mybir.ActivationFunctionType.Exp
    fln = mybir.ActivationFunctionType.Ln
    X = mybir.AxisListType.X
    add = mybir.AluOpType.add

    from contextlib import ExitStack
    ctx = ExitStack()
    with ctx:
        sb = lambda nm, shape: ctx.enter_context(nc.sbuf_tensor(nm, shape, F32))
        sem = lambda nm: ctx.enter_context(nc.semaphore(name=nm))
        tm = [sb("tm_0", [P, F]), sb("tm_1", [P, F])]
        s = [sb("s_0", [P, F]), sb("s_1", [P, F])]
        bn = [sb("bn_0", [P, F]), sb("bn_1", [P, F])]
        g = [sb("g_0", [P, F]), sb("g_1", [P, F])]
        gt = [sb("gt_0", [P, F]), sb("gt_1", [P, F])]
        u = [sb("u_0", [P, F]), sb("u_1", [P, F])]
        eu, tr = sb("eu", [P, F]), sb("tr", [P, F])
        po = sb("po", [P, 5 * BPC])
        bias1 = sb("bias1", [P, 1])
        dtm = [sem("dtm0"), sem("dtm1")]
        ds = [sem("ds0"), sem("ds1")]
        dbn = [sem("dbn0"), sem("dbn1")]
        dg = [sem("dg0"), sem("dg1")]
        dgt = [sem("dgt0"), sem("dgt1")]
        dout, sa, sv, sc = (sem(nm) for nm in ("dout", "sa", "sv", "sc"))
        all_sems = (dtm + ds + dbn + dg + dgt + [dout, sa, sv, sc])
        block = ctx.enter_context(nc.Block(no_gpsimd_drain=True))

        pf = lambda t: t.rearrange("(p f) -> p f", p=P)

        @block.sync
        def _(sync):
            loads = [
                (tm[0], outs[0, 1], dtm[0]),
                (s[0], outs[0, 0], ds[0]),
                (g[0], gts[0], dg[0]),
                (bn[0], outs[0, 2], dbn[0]),
                (tm[1], outs[1, 1], dtm[1]),
                (gt[0], gtt[0], dgt[0]),
                (s[1], outs[1, 0], ds[1]),
                (g[1], gts[1], dg[1]),
                (gt[1], gtt[1], dgt[1]),
                (bn[1], outs[1, 2], dbn[1]),
            ]
            for dst, src, dsem in loads:
                sync.dma_start(out=dst[:, :], in_=pf(src)).then_inc(dsem, 16)
            sync.wait_ge(sa, 7 * BPC)
            sync.wait_ge(sv, 4 * BPC)
            sync.dma_start(out=part[:, :], in_=po[:, :]).then_inc(dout, 16)
            for semh in all_sems:
                if semh is not dout:
                    sync.sem_clear(semh)
            sync.wait_ge(dout, 16)
            sync.sem_clear(dout)

        @block.scalar
        def _(scalar):
            sa_n = 0

            def act(out, in_, func, wait_prev=True, **kw):
                nonlocal sa_n
                if wait_prev and sa_n >= 1:
                    scalar.wait_ge(sa, sa_n)    # write-ack of previous ACT op
                nc.scalar.activation(out=out, in_=in_, func=func,
                                     **kw).then_inc(sa, 1)
                sa_n += 1

            for b in range(BPC):
                # sigmoid(tm) = exp(-ln(1 + exp(-tm))) in place in u[b]
                scalar.wait_ge(dtm[b], 16)
                act(u[b][:, :], tm[b][:, :], fexp, wait_prev=False, scale=-1.0)
                if b == 0:
                    scalar.wait_ge(sc, 1)
                act(u[b][:, :], u[b][:, :], fln, bias=bias1[:, :])
                act(u[b][:, :], u[b][:, :], fexp, scale=-1.0)
                # BCE softplus sums: ln(1 + exp(x)), accumulated per partition
                scalar.wait_ge(ds[b], 16)
                act(eu[:, :], s[b][:, :], fexp)
                act(eu[:, :], eu[:, :], fln, bias=bias1[:, :],
                    accum_out=po[:, 2 * b : 2 * b + 1])
                scalar.wait_ge(dbn[b], 16)
                act(eu[:, :], bn[b][:, :], fexp)
                act(eu[:, :], eu[:, :], fln, bias=bias1[:, :],
                    accum_out=po[:, 2 * b + 1 : 2 * b + 2])
            assert sa_n == 7 * BPC

        @block.vector
        def _(vector):
            nc.vector.memset(bias1[:, :], 1.0).then_inc(sc, 1)
            sv_n = 0

            def stt_sum(b, which):
                # sum (g>0.5)*x for x in {s: col c, bn: col c+1}; writes tr
                nonlocal sv_n
                x, dx, off = ((s, ds, 0) if which == "s" else (bn, dbn, 1))
                if sv_n >= 1:
                    vector.wait_ge(sv, sv_n)   # tr write-ack of previous stt
                vector.wait_ge(dg[b], 16)
                vector.wait_ge(dx[b], 16)
                nc.vector.scalar_tensor_tensor(
                    out=tr[:, :], in0=g[b][:, :], scalar=0.5, in1=x[b][:, :],
                    op0=ag, op1=mul,
                    accum_out=po[:, 4 + 3 * b + off : 5 + 3 * b + off],
                ).then_inc(sv, 1)
                sv_n += 1

            def l1_pair(b):
                # |sigmoid - gt| summed: subtract in place into gt, abs-reduce
                nonlocal sv_n
                vector.wait_ge(sa, 7 * b + 3)   # sigmoid chain done
                vector.wait_ge(dgt[b], 16)
                nc.vector.tensor_tensor(
                    out=gt[b][:, :], in0=u[b][:, :], in1=gt[b][:, :], op=sub
                ).then_inc(sv, 1)
                sv_n += 1
                vector.wait_ge(sv, sv_n)        # subtract write-ack
                nc.vector.tensor_reduce(
                    out=po[:, 6 + 3 * b : 7 + 3 * b], in_=gt[b][:, :],
                    axis=X, op=add, apply_absolute_value=True,
                ).then_inc(sv, 1)
                sv_n += 1

            # image 0: bn arrives before gt; image 1: bn arrives last
            stt_sum(0, "s")
            stt_sum(0, "bn")
            l1_pair(0)
            stt_sum(1, "s")
            l1_pair(1)
            stt_sum(1, "bn")
            assert sv_n == 4 * BPC

    return nc


def _numpy_reference(outputs, gt_shrink_labels, gt_threshold_labels):
    """Exact fallback for inputs outside the fast-path regime."""
    OHEM_RATIO, EPS = 3, 1e-7

    def sigmoid(x):
        return 1.0 / (1.0 + np.exp(-x))

    shrink, thresh, binary = outputs[:, 0], outputs[:, 1], outputs[:, 2]
    b = outputs.shape[0]
    flat_s = shrink.reshape(b, -1)
    flat_pos = (gt_shrink_labels > 0.5).reshape(b, -1)
    n = flat_s.shape[1]
    pos_num = flat_pos.sum(axis=1)
    neg_total = n - pos_num
    neg_num = np.minimum(pos_num * OHEM_RATIO, neg_total)
    neg_scores = np.where(flat_pos, -np.inf, flat_s)
    sorted_desc = -np.sort(-neg_scores, axis=1)
    idx = np.clip(neg_num - 1, 0, n - 1).astype(np.int64)
    thr = np.take_along_axis(sorted_desc, idx[:, None], axis=1)
    mask = (flat_s >= thr) | flat_pos
    valid = (pos_num > 0) & (neg_num > 0)
    mask = (mask & valid[:, None]).reshape(shrink.shape).astype(np.float32)

    def masked_bce(logits, target, m):
        p = np.clip(sigmoid(logits), EPS, 1.0 - EPS)
        t = (target > 0.5).astype(np.float32)
        per_px = -(t * np.log(p) + (1.0 - t) * np.log(1.0 - p))
        denom = m.sum()
        return float(per_px.flatten() @ m.flatten() / max(denom, 1.0)) if denom > 0 else 0.0

    loss_shrink = masked_bce(shrink, gt_shrink_labels, mask)
    loss_binary = masked_bce(binary, gt_shrink_labels, mask)
    m2 = ((gt_threshold_labels > 0) | (gt_shrink_labels > 0)).astype(np.float32)
    denom2 = m2.sum()
    l1 = np.abs(sigmoid(thresh) - gt_threshold_labels).flatten() @ m2.flatten()
    loss_thresh = float(l1 / max(denom2, 1.0)) if denom2 > 0 else 0.0
    loss_all = loss_shrink + ALPHA * loss_binary + BETA * loss_thresh
    return np.array([loss_all, loss_shrink, loss_binary, loss_thresh], np.float32)


def kernel(outputs, gt_shrink_labels, gt_threshold_labels, _trace=False):
    global _CACHED_NC
    outputs = np.ascontiguousarray(np.asarray(outputs, dtype=np.float32))
    gts = np.ascontiguousarray(np.asarray(gt_shrink_labels, dtype=np.float32))
    gtt = np.ascontiguousarray(np.asarray(gt_threshold_labels, dtype=np.float32))

    # ---- host-side regime checks (exactness guards for the fast path) ----
    pos_num = (gts > 0.5).reshape(B, -1).sum(axis=1)
    neg_total = N - pos_num
    neg_num = np.minimum(3 * pos_num, neg_total)
    valid = (pos_num > 0) & (neg_num > 0)
    needs_topk = valid & (3 * pos_num < neg_total)
    clip_active = max(
        float(np.abs(outputs[:, 0]).max()), float(np.abs(outputs[:, 2]).max())
    ) >= 16.0
    if needs_topk.any() or clip_active:
        return _numpy_reference(outputs, gts, gtt)

    if _CACHED_NC is None:
        _CACHED_NC = build_nc()
    nc = _CACHED_NC

    in_maps = []
    for c in range(NCORES):
        sl = slice(c * BPC, (c + 1) * BPC)
        in_maps.append({
            "outs": outputs[sl].reshape(BPC, 3, N),
            "gts": gts[sl].reshape(BPC, N),
            "gtt": gtt[sl].reshape(BPC, N),
        })
    res = run_bass_kernel_spmd(
        nc, in_maps, core_ids=list(range(NCORES)), trace=_trace
    )

    # ---- host combine: per-image sums from per-partition partials ----
    sp_s = np.empty(B); sp_b = np.empty(B); ts = np.empty(B); tb = np.empty(B)
    l1 = np.empty(B)
    for c in range(NCORES):
        po = res.results[c]["part"].astype(np.float64).sum(axis=0)
        for b in range(BPC):
            i = c * BPC + b
            sp_s[i], sp_b[i] = po[2 * b], po[2 * b + 1]
            ts[i], tb[i], l1[i] = po[4 + 3 * b], po[5 + 3 * b], po[6 + 3 * b]

    cnt = float(N * valid.sum())
    num_s = float(((sp_s - ts) * valid).sum())
    num_b = float(((sp_b - tb) * valid).sum())
    loss_shrink = num_s / max(cnt, 1.0) if cnt > 0 else 0.0
    loss_binary = num_b / max(cnt, 1.0) if cnt > 0 else 0.0

    # threshold-loss mask corrections for pixels where both labels <= 0
    zz = (gtt <= 0) & (gts <= 0)
    cnt2 = float(B * N - zz.sum())
    l1_tot = float(l1.sum())
    if zz.any():
        tmz = outputs[:, 1][zz]
        l1_tot -= float(np.abs(1.0 / (1.0 + np.exp(-tmz)) - gtt[zz]).sum())
    loss_thresh = l1_tot / max(cnt2, 1.0) if cnt2 > 0 else 0.0

    loss_all = loss_shrink + ALPHA * loss_binary + BETA * loss_thresh
    out = np.array([loss_all, loss_shrink, loss_binary, loss_thresh], np.float32)
    if _trace:
        return out, res
    return out


# revision 23
# speedup vs baseline: 1.0544x; 1.0474x over previous
"""DBLoss (OHEM-masked BCE + masked L1 threshold loss) on 8 Trainium2 cores.

Shapes are hardcoded for the nn_DBLoss problem:
  outputs             [16, 3, 640, 640] f32
  gt_shrink_labels    [16, 640, 640]    f32
  gt_threshold_labels [16, 640, 640]    f32
Returns np.float32[4] = (loss_all, loss_shrink, loss_binary, loss_thresh).

Sharding: pure data parallel — 2 images per core, 8 cores. Each core computes
per-image partial sums (per-partition [128] vectors); the host reduces the
tiny partials and forms the masked means.

Math notes (device fast path):
 * OHEM: with neg_num == neg_total (i.e. 3*pos_num >= neg_total) the top-k
   threshold is the minimum negative score, so the selection mask is exactly
   all-ones for every valid image. The host verifies this condition per image
   (along with pos_num>0, neg_total>0) and falls back to an exact numpy
   implementation if any image needs a true top-k (cannot happen for the
   problem's uniform-random labels).
 * BCE with binarized target t and no sigmoid clipping reduces to
   softplus(x) - t*x; the host verifies |logits| < 16 so the 1e-7 clip in the
   reference is inactive.
 * threshold-loss mask (gt_t>0)|(gt_s>0): the device sums over all pixels;
   the host subtracts exact corrections for the (measure-zero) pixels where
   both labels are <= 0.
"""

import sys

import numpy as np

try:
    import concourse.bass as bass
except ImportError:  # stand-alone grading dir: fall back to known repo paths
    for _p in ("/root/.axon_site/_ro/trn_rl_repo", "/opt/trn_rl_repo"):
        if _p not in sys.path:
            sys.path.append(_p)
    import concourse.bass as bass

import concourse.tile as tile
from concourse import mybir
from concourse.bass_utils import run_bass_kernel_spmd

B, H, W = 16, 640, 640
N = H * W                    # 409600 pixels / image
P = 128                      # SBUF partitions
F = N // P                   # 3200 free elements / partition
NCORES = 8
BPC = B // NCORES            # 2 images per core
ALPHA, BETA = 1.0, 10.0
F32 = mybir.dt.float32

_CACHED_NC = None


def build_nc() -> "bass.Bass":
    """Per-core raw-bass program.

    Per image: 5 HWDGE channel loads, 7 ACT table ops (exp/ln set only),
    4 big DVE ops; per-partition partial sums in one output tile.

    Raw bass (no TileContext): this walrus build encodes at most ONE attached
    sync-wait per TPB instruction and Tile's kernel-tail drain needs ~10, so
    all cross-engine ordering uses standalone wait_ge instructions
    (EventSemaphore ops, which codegen fine) with explicit semaphores.

    Load order is tuned so ACT (the busiest engine at ~41.4 us of table ops)
    starts after the first 1.6 MB load and never stalls long, and so the
    last-arriving tensors gate the least trailing work:
      tm0 g0 s0 bn0 tm1 gt0 g1 s1 gt1 bn1

    Semaphores: one per input DMA (+16 on completion), sa = ACT op counter
    (then_inc fires on write-ack, so sa>=k also guards same-engine RAW/WAW
    on ACT outputs), sv = DVE op counter, sc = bias-constant memset done,
    dout = output DMA completion. DVE clears every semaphore at the end so
    repeated executions of the loaded NEFF start from zero.
    """
    nc = bass.Bass(dynamic_dma_scratch_size=2048, enable_partition_id=False,
                   monotonic_sem_count=0)
    outs = nc.dram_tensor("outs", [BPC, 3, N], F32, kind="ExternalInput")
    gts = nc.dram_tensor("gts", [BPC, N], F32, kind="ExternalInput")
    gtt = nc.dram_tensor("gtt", [BPC, N], F32, kind="ExternalInput")
    # columns per image b: [2b]=sum softplus(shrink), [2b+1]=sum softplus(bin)
    # then [4+3b]=sum t*shrink, [5+3b]=sum t*bin, [6+3b]=sum|sig-gt|
    part = nc.dram_tensor("part", [P, 12], F32, kind="ExternalOutput")

    ag = mybir.AluOpType.is_gt
    mul = mybir.AluOpType.mult
    sub = mybir.AluOpType.subtract
    fexp = mybir.ActivationFunctionType.Exp
    fln = mybir.ActivationFunctionType.Ln
    X = mybir.AxisListType.X
    add = mybir.AluOpType.add

    from contextlib import ExitStack
    ctx = ExitStack()
    with ctx:
        sb = lambda nm, shape: ctx.enter_context(nc.sbuf_tensor(nm, shape, F32))
        sem = lambda nm: ctx.enter_context(nc.semaphore(name=nm))
        tm = [sb("tm_0", [P, F]), sb("tm_1", [P, F])]
        s = [sb("s_0", [P, F]), sb("s_1", [P, F])]
        bn = [sb("bn_0", [P, F]), sb("bn_1", [P, F])]
        g = [sb("g_0", [P, F]), sb("g_1", [P, F])]
        gt = [sb("gt_0", [P, F]), sb("gt_1", [P, F])]
        u = [sb("u_0", [P, F]), sb("u_1", [P, F])]
        eu, tr = sb("eu", [P, F]), sb("tr", [P, F])
        po = sb("po", [P, 12])
        bias1 = sb("bias1", [P, 1])
        dtm = [sem("dtm0"), sem("dtm1")]
        ds = [sem("ds0"), sem("ds1")]
        dbn = [sem("dbn0"), sem("dbn1")]
        dbnb = sem("dbnb")
        dg = [sem("dg0"), sem("dg1")]
        dgt = [sem("dgt0"), sem("dgt1")]
        dout, sa, sv, sc = (sem(nm) for nm in ("dout", "sa", "sv", "sc"))
        all_sems = (dtm + ds + dbn + dg + dgt + [dbnb, dout, sa, sv, sc])
        block = ctx.enter_context(nc.Block(no_gpsimd_drain=True))

        pf = lambda t: t.rearrange("(p f) -> p f", p=P)

        @block.sync
        def _(sync):
            loads = [
                (tm[0], outs[0, 1], dtm[0]),
                (s[0], outs[0, 0], ds[0]),
                (g[0], gts[0], dg[0]),
                (bn[0], outs[0, 2], dbn[0]),
                (tm[1], outs[1, 1], dtm[1]),
                (gt[0], gtt[0], dgt[0]),
                (s[1], outs[1, 0], ds[1]),
                (g[1], gts[1], dg[1]),
                (gt[1], gtt[1], dgt[1]),
            ]
            for dst, src, dsem in loads:
                sync.dma_start(out=dst[:, :], in_=pf(src)).then_inc(dsem, 16)
            h = F // 2
            bn1f = pf(outs[1, 2])
            sync.dma_start(out=bn[1][:, :h], in_=bn1f[:, :h]).then_inc(dbn[1], 16)
            sync.dma_start(out=bn[1][:, h:], in_=bn1f[:, h:]).then_inc(dbnb, 16)
            sync.wait_ge(sa, 7 * BPC + 2)
            sync.wait_ge(sv, 4 * BPC + 1)
            sync.dma_start(out=part[:, :], in_=po[:, :]).then_inc(dout, 16)
            for semh in all_sems:
                if semh is not dout:
                    sync.sem_clear(semh)
            sync.wait_ge(dout, 16)
            sync.sem_clear(dout)

        @block.scalar
        def _(scalar):
            sa_n = 0

            def act(out, in_, func, wait_prev=True, **kw):
                nonlocal sa_n
                if wait_prev and sa_n >= 1:
                    scalar.wait_ge(sa, sa_n)    # write-ack of previous ACT op
                nc.scalar.activation(out=out, in_=in_, func=func,
                                     **kw).then_inc(sa, 1)
                sa_n += 1

            for b in range(BPC):
                # sigmoid(tm) = exp(-ln(1 + exp(-tm))) in place in u[b]
                scalar.wait_ge(dtm[b], 16)
                act(u[b][:, :], tm[b][:, :], fexp, wait_prev=False, scale=-1.0)
                if b == 0:
                    scalar.wait_ge(sc, 1)
                act(u[b][:, :], u[b][:, :], fln, bias=bias1[:, :])
                act(u[b][:, :], u[b][:, :], fexp, scale=-1.0)
                # BCE softplus sums: ln(1 + exp(x)), accumulated per partition
                scalar.wait_ge(ds[b], 16)
                act(eu[:, :], s[b][:, :], fexp)
                act(eu[:, :], eu[:, :], fln, bias=bias1[:, :],
                    accum_out=po[:, 2 * b : 2 * b + 1])
                if b == 0:
                    scalar.wait_ge(dbn[b], 16)
                    act(eu[:, :], bn[b][:, :], fexp)
                    act(eu[:, :], eu[:, :], fln, bias=bias1[:, :],
                        accum_out=po[:, 1:2])
                else:
                    # bn1 arrives last: process halves as they land
                    h = F // 2
                    scalar.wait_ge(dbn[b], 16)
                    act(eu[:, :h], bn[b][:, :h], fexp)
                    act(eu[:, :h], eu[:, :h], fln, bias=bias1[:, :],
                        accum_out=po[:, 3:4])
                    scalar.wait_ge(dbnb, 16)
                    act(eu[:, h:], bn[b][:, h:], fexp)
                    act(eu[:, h:], eu[:, h:], fln, bias=bias1[:, :],
                        accum_out=po[:, 4:5])
            assert sa_n == 7 * BPC + 2

        @block.vector
        def _(vector):
            nc.vector.memset(bias1[:, :], 1.0).then_inc(sc, 1)
            sv_n = 0

            def stt_sum(b, which, half=None):
                # sum (g>0.5)*x; writes (a slice of) tr
                nonlocal sv_n
                h = F // 2
                cols = {(0, "s"): 5, (0, "bn"): 6, (1, "s"): 8,
                        (1, "bn", 0): 9, (1, "bn", 1): 10}
                if half is None:
                    col = cols[(b, which)]
                    sl = slice(None)
                    dsem = ds[b] if which == "s" else dbn[b]
                else:
                    col = cols[(b, which, half)]
                    sl = slice(0, h) if half == 0 else slice(h, F)
                    dsem = dbn[b] if half == 0 else dbnb
                x = s if which == "s" else bn
                if sv_n >= 1:
                    vector.wait_ge(sv, sv_n)   # tr write-ack of previous op
                vector.wait_ge(dg[b], 16)
                vector.wait_ge(dsem, 16)
                nc.vector.scalar_tensor_tensor(
                    out=tr[:, sl], in0=g[b][:, sl], scalar=0.5,
                    in1=x[b][:, sl], op0=ag, op1=mul,
                    accum_out=po[:, col : col + 1],
                ).then_inc(sv, 1)
                sv_n += 1

            def l1_pair(b):
                # |sigmoid - gt| summed: subtract in place into gt, abs-reduce
                nonlocal sv_n
                vector.wait_ge(sa, 7 * b + 3)   # sigmoid chain done
                vector.wait_ge(dgt[b], 16)
                nc.vector.tensor_tensor(
                    out=gt[b][:, :], in0=u[b][:, :], in1=gt[b][:, :], op=sub
                ).then_inc(sv, 1)
                sv_n += 1
                vector.wait_ge(sv, sv_n)        # subtract write-ack
                nc.vector.tensor_reduce(
                    out=po[:, 7 + 4 * b : 8 + 4 * b], in_=gt[b][:, :],
                    axis=X, op=add, apply_absolute_value=True,
                ).then_inc(sv, 1)
                sv_n += 1

            # image 0: bn arrives before gt; image 1: bn arrives last, halved
            stt_sum(0, "s")
            stt_sum(0, "bn")
            l1_pair(0)
            stt_sum(1, "s")
            l1_pair(1)
            stt_sum(1, "bn", half=0)
            stt_sum(1, "bn", half=1)
            assert sv_n == 4 * BPC + 1

    return nc


def _numpy_reference(outputs, gt_shrink_labels, gt_threshold_labels):
    """Exact fallback for inputs outside the fast-path regime."""
    OHEM_RATIO, EPS = 3, 1e-7

    def sigmoid(x):
        return 1.0 / (1.0 + np.exp(-x))

    shrink, thresh, binary = outputs[:, 0], outputs[:, 1], outputs[:, 2]
    b = outputs.shape[0]
    flat_s = shrink.reshape(b, -1)
    flat_pos = (gt_shrink_labels > 0.5).reshape(b, -1)
    n = flat_s.shape[1]
    pos_num = flat_pos.sum(axis=1)
    neg_total = n - pos_num
    neg_num = np.minimum(pos_num * OHEM_RATIO, neg_total)
    neg_scores = np.where(flat_pos, -np.inf, flat_s)
    sorted_desc = -np.sort(-neg_scores, axis=1)
    idx = np.clip(neg_num - 1, 0, n - 1).astype(np.int64)
    thr = np.take_along_axis(sorted_desc, idx[:, None], axis=1)
    mask = (flat_s >= thr) | flat_pos
    valid = (pos_num > 0) & (neg_num > 0)
    mask = (mask & valid[:, None]).reshape(shrink.shape).astype(np.float32)

    def masked_bce(logits, target, m):
        p = np.clip(sigmoid(logits), EPS, 1.0 - EPS)
        t = (target > 0.5).astype(np.float32)
        per_px = -(t * np.log(p) + (1.0 - t) * np.log(1.0 - p))
        denom = m.sum()
        return float(per_px.flatten() @ m.flatten() / max(denom, 1.0)) if denom > 0 else 0.0

    loss_shrink = masked_bce(shrink, gt_shrink_labels, mask)
    loss_binary = masked_bce(binary, gt_shrink_labels, mask)
    m2 = ((gt_threshold_labels > 0) | (gt_shrink_labels > 0)).astype(np.float32)
    denom2 = m2.sum()
    l1 = np.abs(sigmoid(thresh) - gt_threshold_labels).flatten() @ m2.flatten()
    loss_thresh = float(l1 / max(denom2, 1.0)) if denom2 > 0 else 0.0
    loss_all = loss_shrink + ALPHA * loss_binary + BETA * loss_thresh
    return np.array([loss_all, loss_shrink, loss_binary, loss_thresh], np.float32)


def kernel(outputs, gt_shrink_labels, gt_threshold_labels, _trace=False):
    global _CACHED_NC
    outputs = np.ascontiguousarray(np.asarray(outputs, dtype=np.float32))
    gts = np.ascontiguousarray(np.asarray(gt_shrink_labels, dtype=np.float32))
    gtt = np.ascontiguousarray(np.asarray(gt_threshold_labels, dtype=np.float32))

    # ---- host-side regime checks (exactness guards for the fast path) ----
    pos_num = (gts > 0.5).reshape(B, -1).sum(axis=1)
    neg_total = N - pos_num
    neg_num = np.minimum(3 * pos_num, neg_total)
    valid = (pos_num > 0) & (neg_num > 0)
    needs_topk = valid & (3 * pos_num < neg_total)
    clip_active = max(
        float(np.abs(outputs[:, 0]).max()), float(np.abs(outputs[:, 2]).max())
    ) >= 16.0
    if needs_topk.any() or clip_active:
        return _numpy_reference(outputs, gts, gtt)

    if _CACHED_NC is None:
        _CACHED_NC = build_nc()
    nc = _CACHED_NC

    in_maps = []
    for c in range(NCORES):
        sl = slice(c * BPC, (c + 1) * BPC)
        in_maps.append({
            "outs": outputs[sl].reshape(BPC, 3, N),
            "gts": gts[sl].reshape(BPC, N),
            "gtt": gtt[sl].reshape(BPC, N),
        })
    res = run_bass_kernel_spmd(
        nc, in_maps, core_ids=list(range(NCORES)), trace=_trace
    )

    # ---- host combine: per-image sums from per-partition partials ----
    sp_s = np.empty(B); sp_b = np.empty(B); ts = np.empty(B); tb = np.empty(B)
    l1 = np.empty(B)
    for c in range(NCORES):
        po = res.results[c]["part"].astype(np.float64).sum(axis=0)
        i0, i1 = c * BPC, c * BPC + 1
        sp_s[i0], sp_b[i0] = po[0], po[1]
        sp_s[i1], sp_b[i1] = po[2], po[3] + po[4]
        ts[i0], tb[i0], l1[i0] = po[5], po[6], po[7]
        ts[i1], tb[i1], l1[i1] = po[8], po[9] + po[10], po[11]

    cnt = float(N * valid.sum())
    num_s = float(((sp_s - ts) * valid).sum())
    num_b = float(((sp_b - tb) * valid).sum())
    loss_shrink = num_s / max(cnt, 1.0) if cnt > 0 else 0.0
    loss_binary = num_b / max(cnt, 1.0) if cnt > 0 else 0.0

    # threshold-loss mask corrections for pixels where both labels <= 0
    zz = (gtt <= 0) & (gts <= 0)
    cnt2 = float(B * N - zz.sum())
    l1_tot = float(l1.sum())
    if zz.any():
        tmz = outputs[:, 1][zz]
        l1_tot -= float(np.abs(1.0 / (1.0 + np.exp(-tmz)) - gtt[zz]).sum())
    loss_thresh = l1_tot / max(cnt2, 1.0) if cnt2 > 0 else 0.0

    loss_all = loss_shrink + ALPHA * loss_binary + BETA * loss_thresh
    out = np.array([loss_all, loss_shrink, loss_binary, loss_thresh], np.float32)
    if _trace:
        return out, res
    return out
